# revision 1
# baseline (speedup 1.0000x reference)
"""Trainium2 Bass kernel for nn_AttnLoss_26551487823965.

Computes the attention MSE loss:
  loss = sum_c sgn_c * (cross_mse_c + sum_j gate[c,j] * Lj[j])
where Lj = mean over hw pixels of (self_attn - self_attn_erase)^2 per
pixel-column, gate = thresholded bilinear-resized attention masks, and
cross_mse = per-token-column MSE of attn vs attn_erase.

Sharding: self_attn/self_attn_erase are split along the last (hw=4096)
axis into 512 columns per core (8 cores). Each core computes its slice's
column sums of (x-y)^2, the full mask gates (cheap, needs only
attn[:,:,1:9]), and the full cross term; the host sums the per-core
partial scalars.

Structure per core (all inside one Tile program):
 - cross-attn MSE + per-channel mask pipeline (img -> transpose ->
   r = img@U^T -> up = U@r -> max -> threshold bits -> M1 -> mask ->
   gates) emitted FIRST so it overlaps the DMA-bound streaming loop;
   per-channel small PSUM tiles are packed into one bank-slot tile
   (ps_pk) so three channels can be in flight.
 - streaming loop: 4 x/y tile pairs of 2 MB, DMAs round-robined over
   the three rings (qSP-HWDGE, qAct-HWDGE, qPool-SWDGE); subtract on
   DVE/Pool alternating, squares on ACT/Pool alternating writing
   float32r, column sums accumulated by 32 ones-weight f32r matmuls
   into one PSUM row.
 - tail: S1 row reshaped to an (8,64) grid via one SBUF->SBUF DMA,
   gate-weighted dot via PE, two scalars DMA'd out.
A post-build legalization pass (_legalize_waits) splits multi-sync-wait
instructions into single-wait NoOp chains because this container's
walrus rejects them (see memory notes).

Key observations used:
 - Only columns 1..8 of attn/attn_erase matter (PROMPT_N=10, token
   channels 1..8); the other 69 columns are dead.
 - The 255/max rescale before the threshold comparison cancels (all
   values nonnegative), and binary {0,255} vs {0,1} is irrelevant
   because only mask>0 is consumed.  Verified margins: min
   |up - thr*max|/max ~ 4e-6 across channels, ~40x the fp32 noise
   between implementations.
 - Bilinear resize is separable: up = U @ img @ U^T with exact
   (binary-fraction) weights; the 256->64 downsample weights are
   exactly {0, 0.5}, so the mask values and the 64->256 matrices are
   exact; mask-side matmuls can run in bf16 exactly.
"""

from contextlib import ExitStack

import numpy as np

H = 64
W = 64
HW = H * W                      # 4096 pixels = mean axis
SEQ = 77
NCORES = 8
COLS = HW // NCORES             # 512 pixel-columns per core
C = 8                           # prompt token channels (seq idx 1..8)
UPS = 256
NT = 4                          # pixel tiles per core
SUB = 8                         # 128-row subtiles per pixel tile
TOKEN_CHANNELS = (1, 4)         # c with seq idx in TOKEN_INDICES=(2,5); c = idx-1
THRS = [0.85 if c in TOKEN_CHANNELS else 0.95 for c in range(C)]
SGNS = [-1.0 if c in TOKEN_CHANNELS else 1.0 for c in range(C)]

# column layout of the merged f32 constant tensor (128 partitions)
CF_IDENT = 0          # [0:128]   identity (128,128)
CF_UT = 128           # [128:384] U^T rows 0:64        (64,256)
CF_UHT = 384          # [384:640] U-halves^T rows 0:64 (64,2,128)
CF_ONES = 640         # [640:641] ones column (128,1)
CF_ONESR = 641        # [641:769] ones row (1,128)
CF_SGN = 769          # [769:777] sign row (1,8)
CF_W = 777
# column layout of the merged bf16 constant tensor
CB_DYT = 0            # [0:16]    Dy_slice^T halves (128,2,8)
CB_DXT = 16           # [16:144]  Dx^T halves (128,2,64)
CB_IDENT = 144        # [144:152] identity rows 0:8 (8,8)
CB_W = 152

_PROG_CACHE = {}


def _interp_matrix(out_n, in_n):
    """Row-interpolation matrix of torch bilinear resize (align_corners=False).

    All weights are exact binary fractions for (256,64) and (64,256)."""
    ys = np.clip(
        (np.arange(out_n, dtype=np.float64) + 0.5) * (in_n / out_n) - 0.5,
        0.0, in_n - 1.0,
    )
    y0 = np.floor(ys).astype(np.int64)
    y1 = np.minimum(y0 + 1, in_n - 1)
    wy = ys - y0
    m = np.zeros((out_n, in_n), dtype=np.float32)
    np.add.at(m, (np.arange(out_n), y0), (1.0 - wy).astype(np.float32))
    np.add.at(m, (np.arange(out_n), y1), wy.astype(np.float32))
    return m


def _legalize_waits(nc):
    """Split multi-wait instructions into single-wait NoOp prefixes.

    The walrus build in this container rejects instructions whose ISA
    struct cannot hold all the sync waits Tile assigned (fp32 matmul
    LDWEIGHTS holds one, several v2-lowered structs hold one, the
    kernel-tail Drain holds few).  Engine queues execute in order, so
    hoisting extra waits onto same-engine NoOps (the same pattern the
    all-engine-barrier drains use) preserves semantics.  Matmults
    additionally must not carry DMA-queue-sem waits at all."""
    import concourse.mybir as mybir
    import re

    _MONO_SEM = re.compile(r"^(Pool|Activation|PE|DVE|SP|DMAHW\d|DMASW\d)_\d+$")
    n = 0
    seen = {}  # (engine, sem id) -> max value already waited on that engine
    for f in nc.m.functions:
        for b in f.blocks:
            insts = b.instructions
            out = []
            for inst in insts:
                si = inst.sync_info
                waits = list(si.on_wait) if si and si.on_wait else []
                # drop waits dominated by an earlier same-engine wait
                # (engines execute their queue in order, so sem >= v' with
                # v' >= v implies sem >= v)
                if waits:
                    eng = inst.engine
                    kept = []
                    changed = False
                    for w in waits:
                        kk = (eng, w.id)
                        monotone = bool(_MONO_SEM.match(w.ant_name or ""))
                        if (monotone
                                and getattr(w, "wait_mode", "") == "sem-ge-imm"
                                and w.wait_value is not None
                                and seen.get(kk, -1) >= w.wait_value):
                            changed = True
                            continue
                        kept.append(w)
                        if (monotone
                                and getattr(w, "wait_mode", "") == "sem-ge-imm"
                                and w.wait_value is not None):
                            seen[kk] = max(seen.get(kk, -1), w.wait_value)
                    if changed:
                        inst.sync_info = mybir.SyncInfo(
                            on_wait=kept, on_update=list(si.on_update or []))
                        si = inst.sync_info
                    waits = kept
                is_mm = type(inst).__name__ == "InstMatmult"
                mm_dma = is_mm and any(
                    "DMA" in (w.ant_name or "") for w in waits)
                keep, move = waits, []
                if len(waits) > 1 or mm_dma:
                    eng_w = [w for w in waits if "DMA" not in (w.ant_name or "")]
                    if eng_w:
                        keep = [eng_w[-1]]
                        move = [w for w in waits if w is not keep[0]]
                    else:
                        keep = []
                        move = waits
                if move:
                    for w in move:
                        nop = mybir.InstNoOp(
                            name=f"{inst.name}-lw{n}", ins=[], outs=[],
                            engine=inst.engine)
                        nop.sync_info = mybir.SyncInfo(on_wait=[w], on_update=[])
                        nc.register_instruction(nop)
                        out.append(nop)
                        n += 1
                    inst.sync_info = mybir.SyncInfo(
                        on_wait=keep, on_update=list(si.on_update or []))
                out.append(inst)
            insts[:] = out
    return nc


def _build_program_legalized(repeat=1):
    return _legalize_waits(_build_program_raw(repeat))


def _build_program_raw(repeat=1):
    import concourse.bass as bass
    import concourse.mybir as mybir
    import concourse.tile as tile

    f32 = mybir.dt.float32
    bf16 = mybir.dt.bfloat16
    OP = mybir.AluOpType
    AF = mybir.ActivationFunctionType

    nc = bass.Bass()

    x = nc.dram_tensor("x", [HW, COLS], f32, kind="ExternalInput")
    y = nc.dram_tensor("y", [HW, COLS], f32, kind="ExternalInput")
    ae8 = nc.dram_tensor("ae8", [H, 2 * W * C], f32, kind="ExternalInput")
    a8T = nc.dram_tensor("a8T", [W, H * C], f32, kind="ExternalInput")
    cf = nc.dram_tensor("cf", [128, CF_W], f32, kind="ExternalInput")
    cb = nc.dram_tensor("cb", [128, CB_W], bf16, kind="ExternalInput")
    out = nc.dram_tensor("out", [1, 2 * repeat], f32, kind="ExternalOutput")

    with tile.TileContext(nc) as tc, ExitStack() as ctx:
        consts = ctx.enter_context(tc.tile_pool(name="consts", bufs=1))
        xyp = ctx.enter_context(tc.tile_pool(name="xyp", bufs=4))
        dpool = ctx.enter_context(tc.tile_pool(name="dpool", bufs=2))
        spool = ctx.enter_context(tc.tile_pool(name="spool", bufs=2))
        small = ctx.enter_context(tc.tile_pool(name="small", bufs=2))
        acc = ctx.enter_context(tc.tile_pool(name="acc", bufs=1))
        ps_s1 = ctx.enter_context(tc.tile_pool(name="ps_s1", bufs=1, space="PSUM"))
        ps_up = ctx.enter_context(tc.tile_pool(name="ps_up", bufs=2, space="PSUM"))
        ps_sm = ctx.enter_context(tc.tile_pool(name="ps_sm", bufs=1, space="PSUM"))
        ps_pk = ctx.enter_context(tc.tile_pool(name="ps_pk", bufs=3, space="PSUM"))
        ps_mk = ctx.enter_context(tc.tile_pool(name="ps_mk", bufs=1, space="PSUM"))

        cf_sb = consts.tile([128, CF_W], f32)
        nc.gpsimd.dma_start(out=cf_sb, in_=cf[:, :])
        cb_sb = consts.tile([128, CB_W], bf16)
        nc.gpsimd.dma_start(out=cb_sb, in_=cb[:, :])

        identf_sb = cf_sb[:, CF_IDENT:CF_IDENT + 128]
        ut_sb = cf_sb[0:H, CF_UT:CF_UT + UPS]
        uht_sb = cf_sb[0:H, CF_UHT:CF_UHT + UPS].rearrange(
            "k (h m) -> k h m", h=2)
        ones_sb = cf_sb[:, CF_ONES:CF_ONES + 1]
        onesr_sb = cf_sb[0:1, CF_ONESR:CF_ONESR + 128]
        sgn_sb = cf_sb[0:1, CF_SGN:CF_SGN + C]
        dyt_sb = cb_sb[:, CB_DYT:CB_DYT + 2 * C].rearrange(
            "k (h m) -> k h m", h=2)
        dxt_sb = cb_sb[:, CB_DXT:CB_DXT + 2 * W].rearrange(
            "k (h m) -> k h m", h=2)
        identb_sb = cb_sb[0:C, CB_IDENT:CB_IDENT + C]

        ones_r = consts.tile([128, 1], mybir.dt.float32r)
        nc.scalar.copy(ones_r, ones_sb)

        for rep in range(repeat):
            # ---- cross-attn term (identical on every core) ----
            ae8t = small.tile([H, 2 * W * C], f32, tag="ae8t")
            nc.gpsimd.dma_start(out=ae8t, in_=ae8[:, :])
            a8tt = small.tile([W, H * C], f32, tag="a8tt")
            nc.gpsimd.dma_start(out=a8tt, in_=a8T[:, :])
            d8 = small.tile([H, W * C], f32, tag="d8")
            nc.vector.tensor_tensor(
                d8, ae8t[:, 0:W * C], ae8t[:, W * C:2 * W * C], OP.subtract)
            s8 = small.tile([H, W * C], f32, tag="s8")
            nc.scalar.activation(s8, d8, AF.Square)
            cross_ps = ps_sm.tile([1, W * C], f32, tag="ps")
            nc.tensor.matmul(
                cross_ps, lhsT=ones_sb[0:H, :], rhs=s8, start=True, stop=True)
            cross8 = small.tile([1, C], f32, tag="cross8")
            nc.vector.reduce_sum(
                out=cross8,
                in_=cross_ps.rearrange("p (j c) -> p c j", c=C),
                axis=mybir.AxisListType.X,
            )
            crossw = small.tile([1, C], f32, tag="crossw")
            nc.vector.tensor_tensor(crossw, cross8, sgn_sb, OP.mult)
            cross1 = small.tile([1, 1], f32, tag="cross1")
            nc.vector.reduce_sum(out=cross1, in_=crossw, axis=mybir.AxisListType.X)

            # ---- mask pipeline: per-channel upsample + threshold ----
            b01 = acc.tile([128, C * 2 * UPS], bf16, tag="b01")  # [c, h, l]
            m1_sb = acc.tile([C, C * UPS], bf16, tag="m1")
            m1t_sb = acc.tile([128, 2 * C * C], bf16, tag="m1t")
            mask_ps = ps_mk.tile([C, C * W], f32, tag="mask")
            g_sb = acc.tile([C, C * W], f32, tag="g")
            w8 = acc.tile([C, W], f32, tag="w8")
            ae8_cj = ae8t[:, 0:W * C].rearrange("i (j c) -> i c j", c=C)
            a8t_ci = a8tt.rearrange("j (i c) -> j c i", c=C)
            for c in range(C):
                pk = ps_pk.tile([128, 512], f32, tag="pk")
                r_ps = pk[0:H, 64:320]
                nc.tensor.matmul(
                    r_ps, lhsT=a8t_ci[:, c, :], rhs=ut_sb,
                    start=True, stop=True)
                r_sb = small.tile([H, UPS], f32, tag="r")
                nc.scalar.copy(r_sb, r_ps)
                up_ps = ps_up.tile([128, 2 * UPS], f32, tag="up")
                for h in range(2):
                    nc.tensor.matmul(
                        up_ps[:, h * UPS:(h + 1) * UPS],
                        lhsT=uht_sb[:, h, :], rhs=r_sb, start=True, stop=True,
                    )
                mxc = small.tile([128, 1], f32, tag="mxc")
                nc.vector.reduce_max(out=mxc, in_=up_ps, axis=mybir.AxisListType.X)
                mxr_ps = pk[0:1, 320:448]
                nc.tensor.transpose(mxr_ps, mxc, identf_sb)
                mxs = small.tile([1, 1], f32, tag="mxs")
                nc.vector.reduce_max(out=mxs, in_=mxr_ps, axis=mybir.AxisListType.X)
                ts_sb = small.tile([1, 1], f32, tag="ts")
                nc.vector.tensor_scalar_mul(ts_sb, in0=mxs, scalar1=float(THRS[c]))
                tb_ps = pk[0:128, 448:449]
                nc.tensor.matmul(
                    tb_ps, lhsT=onesr_sb, rhs=ts_sb, start=True, stop=True)
                tthr = small.tile([128, 1], f32, tag="tthr")
                nc.vector.tensor_copy(tthr, tb_ps)
                nc.vector.tensor_scalar(
                    out=b01[:, c * 2 * UPS:(c + 1) * 2 * UPS],
                    in0=up_ps,
                    scalar1=tthr, scalar2=None, op0=OP.is_ge,
                )
                m1_ps = ps_sm.tile([C, UPS], f32, tag="ps")
                for h in range(2):
                    nc.tensor.matmul(
                        m1_ps, lhsT=dyt_sb[:, h, :],
                        rhs=b01[:, c * 2 * UPS + h * UPS:
                                c * 2 * UPS + (h + 1) * UPS],
                        start=(h == 0), stop=(h == 1),
                    )
                nc.vector.tensor_copy(m1_sb[:, c * UPS:(c + 1) * UPS], m1_ps)
                for h in range(2):
                    tr_ps = ps_sm.tile([128, C], bf16, tag="ps")
                    nc.tensor.transpose(
                        tr_ps,
                        m1_sb[0:C, c * UPS + h * 128: c * UPS + (h + 1) * 128],
                        identb_sb,
                    )
                    k = (2 * c + h) * C
                    nc.vector.tensor_copy(m1t_sb[:, k:k + C], tr_ps)
                for h in range(2):
                    k = (2 * c + h) * C
                    nc.tensor.matmul(
                        mask_ps[:, c * W:(c + 1) * W],
                        lhsT=m1t_sb[:, k:k + C],
                        rhs=dxt_sb[:, h, :],
                        start=(h == 0), stop=(h == 1),
                    )
                nc.vector.tensor_scalar(
                    out=g_sb[:, c * W:(c + 1) * W],
                    in0=mask_ps[:, c * W:(c + 1) * W],
                    scalar1=0.0, scalar2=None, op0=OP.is_gt)
                if c == 0:
                    nc.gpsimd.tensor_copy(w8, g_sb[:, 0:W])
                else:
                    nc.gpsimd.tensor_tensor(
                        w8, w8, g_sb[:, c * W:(c + 1) * W],
                        OP.add if SGNS[c] > 0 else OP.subtract)

            # ---- main streaming loop: column sums of (x-y)^2 ----
            s1 = ps_s1.tile([1, COLS], f32, tag="s1")
            n_mm = NT * SUB
            mm = 0
            rows = HW // NT
            rings = [nc.sync, nc.scalar, nc.gpsimd]
            ring_i = 0
            for t in range(NT):
                xt = xyp.tile([128, SUB, COLS], f32, tag="xt")
                yt = xyp.tile([128, SUB, COLS], f32, tag="yt")
                rings[ring_i % 3].dma_start(
                    out=xt, in_=x[t * rows:(t + 1) * rows, :].rearrange(
                        "(s q) j -> q s j", q=128))
                ring_i += 1
                rings[ring_i % 3].dma_start(
                    out=yt, in_=y[t * rows:(t + 1) * rows, :].rearrange(
                        "(s q) j -> q s j", q=128))
                ring_i += 1
                st = spool.tile([128, SUB, COLS], mybir.dt.float32r, tag="s")
                for q in range(0, SUB, 2):
                    sub_eng = nc.vector if (t * SUB + q) % 8 < 2 else nc.gpsimd
                    sub_eng.tensor_tensor(
                        xt[:, q:q + 2, :], xt[:, q:q + 2, :],
                        yt[:, q:q + 2, :], OP.subtract)
                    if (t * SUB + q) % 8 < 4:
                        nc.scalar.activation(
                            st[:, q:q + 2, :], xt[:, q:q + 2, :], AF.Square)
                    else:
                        nc.gpsimd.tensor_tensor(
                            st[:, q:q + 2, :], xt[:, q:q + 2, :],
                            xt[:, q:q + 2, :], OP.mult)
                    for s in range(q, q + 2):
                        nc.tensor.matmul(
                            s1, lhsT=ones_r,
                            rhs=st[:, s, :],
                            start=(mm == 0), stop=(mm == n_mm - 1),
                            skip_group_check=True,
                        )
                        mm += 1

            # ---- dot with column sums (gates accumulated per channel) ----
            s1_sb = acc.tile([1, COLS], f32, tag="s1sb")
            nc.vector.tensor_copy(s1_sb, s1)
            lg8 = acc.tile([C, W], f32, tag="lg8")
            nc.scalar.dma_start(out=lg8, in_=s1_sb)
            p8 = acc.tile([C, W], f32, tag="p8")
            nc.gpsimd.tensor_tensor(p8, w8, lg8, OP.mult)
            self_ps = ps_sm.tile([1, W], f32, tag="ps")
            nc.tensor.matmul(
                self_ps, lhsT=ones_sb[0:C, :], rhs=p8, start=True, stop=True)
            selfs = acc.tile([1, 1], f32, tag="selfs")
            nc.vector.reduce_sum(out=selfs, in_=self_ps, axis=mybir.AxisListType.X)

            out_sb = acc.tile([1, 2], f32, tag="outsb")
            nc.scalar.copy(out_sb[:, 0:1], selfs)
            nc.scalar.copy(out_sb[:, 1:2], cross1)
            nc.scalar.dma_start(out=out[:, 2 * rep:2 * rep + 2], in_=out_sb)

    return nc


def _build_program(repeat=1):
    return _build_program_legalized(repeat)


def _get_program(repeat=1):
    key = ("nc", repeat)
    if key not in _PROG_CACHE:
        _PROG_CACHE[key] = _build_program(repeat)
    return _PROG_CACHE[key]


def _host_constants():
    if "consts" in _PROG_CACHE:
        return _PROG_CACHE["consts"]
    from concourse import mybir

    np_bf16 = mybir.dt.np(mybir.dt.bfloat16)
    u = _interp_matrix(UPS, H)          # (256, 64) upsample
    d = _interp_matrix(W, UPS)          # (64, 256) downsample, weights in {0,.5}

    cfh = np.zeros((128, CF_W), dtype=np.float32)
    cfh[:, CF_IDENT:CF_IDENT + 128] = np.eye(128, dtype=np.float32)
    cfh[0:H, CF_UT:CF_UT + UPS] = u.T
    cfh[0:H, CF_UHT:CF_UHT + UPS] = u.T.reshape(H, UPS)  # same data as ut;
    # uht view k,(h m): U[h*128+m, k] == u.T[k, h*128+m] -> identical layout
    cfh[:, CF_ONES] = 1.0
    cfh[0, CF_ONESR:CF_ONESR + 128] = 1.0
    cfh[0, CF_SGN:CF_SGN + C] = np.asarray(SGNS, dtype=np.float32)

    cbs = []
    dxt = d.T.reshape(2, 128, W).transpose(1, 0, 2).reshape(128, 2 * W)
    for core in range(NCORES):
        cbh = np.zeros((128, CB_W), dtype=np.float32)
        dsl = d[core * C:(core + 1) * C, :]  # (8, 256)
        cbh[:, CB_DYT:CB_DYT + 2 * C] = (
            dsl.T.reshape(2, 128, C).transpose(1, 0, 2).reshape(128, 2 * C))
        cbh[:, CB_DXT:CB_DXT + 2 * W] = dxt
        cbh[0:C, CB_IDENT:CB_IDENT + C] = np.eye(C, dtype=np.float32)
        cbs.append(np.ascontiguousarray(cbh).astype(np_bf16))

    consts = {"cf": cfh, "cbs": cbs}
    _PROG_CACHE["consts"] = consts
    return consts


def _make_in_maps(inputs):
    attn = np.ascontiguousarray(inputs["attn"], dtype=np.float32)
    attn_erase = np.ascontiguousarray(inputs["attn_erase"], dtype=np.float32)
    sa = np.ascontiguousarray(
        inputs["self_attn"], dtype=np.float32).reshape(HW, HW)
    sae = np.ascontiguousarray(
        inputs["self_attn_erase"], dtype=np.float32).reshape(HW, HW)

    a8 = np.ascontiguousarray(attn[:, :, 1:1 + C]).reshape(H, W * C)
    a8t_host = np.ascontiguousarray(
        attn[:, :, 1:1 + C].transpose(1, 0, 2)).reshape(W, H * C)
    e8 = np.ascontiguousarray(attn_erase[:, :, 1:1 + C]).reshape(H, W * C)
    ae8 = np.concatenate([a8, e8], axis=1)
    ch = _host_constants()

    in_maps = []
    for core in range(NCORES):
        in_maps.append({
            "x": np.ascontiguousarray(sa[:, core * COLS:(core + 1) * COLS]),
            "y": np.ascontiguousarray(sae[:, core * COLS:(core + 1) * COLS]),
            "ae8": ae8,
            "a8T": a8t_host,
            "cf": ch["cf"],
            "cb": ch["cbs"][core],
        })
    return in_maps


def _combine(outs):
    self_raw = sum(float(o[0, 0]) for o in outs)
    cross_raw = float(outs[0][0, 1])
    return np.float32((self_raw + cross_raw) / float(HW))


def kernel(**inputs):
    from concourse.bass_utils import run_bass_kernel_spmd

    nc = _get_program()
    in_maps = _make_in_maps(inputs)
    res = run_bass_kernel_spmd(nc, in_maps, core_ids=list(range(NCORES)))
    return _combine([r["out"] for r in res.results])



# revision 14
# speedup vs baseline: 1.3545x; 1.3545x over previous
"""Trainium2 Bass kernel for nn_AttnLoss_26551487823965.

Computes the attention MSE loss:
  loss = sum_c sgn_c * (cross_mse_c + sum_j gate[c,j] * Lj[j])
where Lj = mean over hw pixels of (self_attn - self_attn_erase)^2 per
pixel-column, gate = thresholded bilinear-resized attention masks, and
cross_mse = per-token-column MSE of attn vs attn_erase.

Sharding: self_attn/self_attn_erase are split along the last (hw=4096)
axis into 512 columns per core (8 cores). Each core computes its slice's
column sums of (x-y)^2, the full mask gates (cheap, needs only
attn[:,:,1:9]), and the full cross term; the host sums the per-core
partial scalars.

Structure per core (all inside one Tile program):
 - cross-attn MSE + per-channel mask pipeline (img -> transpose ->
   r = img@U^T -> up = U@r -> max -> threshold bits -> M1 -> mask ->
   gates) emitted FIRST so it overlaps the DMA-bound streaming loop;
   per-channel small PSUM tiles are packed into one bank-slot tile
   (ps_pk) so three channels can be in flight.  The signed gate grid
   w8 [8,64] is flattened to wrow [1,512] by a small SBUF->SBUF DMA
   during streaming, so the tail needs no reshape.
 - streaming loop: tapered x/y chunk pairs (SUBS*128 rows each), x
   chunks all on the SP HWDGE ring, y chunks all on the ACT HWDGE
   ring, small/const DMAs on Pool SWDGE.  Partition q of a chunk
   holds SUB consecutive DRAM rows, so every partition is ONE
   contiguous 2048*SUB-byte descriptor (8-22 KB) instead of SUB 2 KB
   descriptors -- much better SDMA efficiency at the ~360 GB/s
   per-core HBM roofline.  Subtract on DVE/Pool alternating, squares
   on ACT writing float32r, column sums accumulated by 32 ones-weight
   f32r matmuls into one PSUM row.  The last chunk is a single 128-row
   block processed split-by-columns across engines so the post-DMA
   drain is minimal.
 - tail: s1w = s1 (*) wrow on DVE (PSUM read), reduce_sum -> scalar,
   one [1,2] out DMA (cross term was reduced into out_sb earlier).
A post-build legalization pass (_legalize_waits) splits multi-sync-wait
instructions into single-wait NoOp chains because this container's
walrus rejects them (see memory notes).

Key observations used:
 - Only columns 1..8 of attn/attn_erase matter (PROMPT_N=10, token
   channels 1..8); the other 69 columns are dead.
 - The 255/max rescale before the threshold comparison cancels (all
   values nonnegative), and binary {0,255} vs {0,1} is irrelevant
   because only mask>0 is consumed.  Verified margins: min
   |up - thr*max|/max ~ 4e-6 across channels, ~40x the fp32 noise
   between implementations.
 - Bilinear resize is separable: up = U @ img @ U^T with exact
   (binary-fraction) weights; the 256->64 downsample weights are
   exactly {0, 0.5}, so the mask values and the 64->256 matrices are
   exact; mask-side matmuls can run in bf16 exactly.
"""

from contextlib import ExitStack

import numpy as np

H = 64
W = 64
HW = H * W                      # 4096 pixels = mean axis
SEQ = 77
NCORES = 8
COLS = HW // NCORES             # 512 pixel-columns per core
C = 8                           # prompt token channels (seq idx 1..8)
UPS = 256
SUBS = [11, 10, 6, 3, 1, 1]     # 128-row blocks per streaming chunk (sum 32);
                                # tapered so every chunk's compute fits in the
                                # stream time remaining after it lands and the
                                # drain after the final DMA is tiny
DRAIN_CHUNKS = 3                # last chunks run column-split across engines
TOKEN_CHANNELS = (1, 4)         # c with seq idx in TOKEN_INDICES=(2,5); c = idx-1
THRS = [0.85 if c in TOKEN_CHANNELS else 0.95 for c in range(C)]
SGNS = [-1.0 if c in TOKEN_CHANNELS else 1.0 for c in range(C)]

# column layout of the merged f32 constant tensor (128 partitions)
CF_IDENT = 0          # [0:128]   identity (128,128)
CF_UT = 128           # [128:384] U^T rows 0:64        (64,256)
CF_UHT = 384          # [384:640] U-halves^T rows 0:64 (64,2,128)
CF_ONES = 640         # [640:641] ones column (128,1)
CF_ONESR = 641        # [641:769] ones row (1,128)
CF_SGN = 769          # [769:777] sign row (1,8)
CF_W = 777
# column layout of the merged bf16 constant tensor
CB_DYT = 0            # [0:16]    Dy_slice^T halves (128,2,8)
CB_DXT = 16           # [16:144]  Dx^T halves (128,2,64)
CB_IDENT = 144        # [144:152] identity rows 0:8 (8,8)
CB_W = 152

_PROG_CACHE = {}


def _interp_matrix(out_n, in_n):
    """Row-interpolation matrix of torch bilinear resize (align_corners=False).

    All weights are exact binary fractions for (256,64) and (64,256)."""
    ys = np.clip(
        (np.arange(out_n, dtype=np.float64) + 0.5) * (in_n / out_n) - 0.5,
        0.0, in_n - 1.0,
    )
    y0 = np.floor(ys).astype(np.int64)
    y1 = np.minimum(y0 + 1, in_n - 1)
    wy = ys - y0
    m = np.zeros((out_n, in_n), dtype=np.float32)
    np.add.at(m, (np.arange(out_n), y0), (1.0 - wy).astype(np.float32))
    np.add.at(m, (np.arange(out_n), y1), wy.astype(np.float32))
    return m


def _legalize_waits(nc):
    """Split multi-wait instructions into single-wait NoOp prefixes.

    The walrus build in this container rejects instructions whose ISA
    struct cannot hold all the sync waits Tile assigned (fp32 matmul
    LDWEIGHTS holds one, several v2-lowered structs hold one, the
    kernel-tail Drain holds few).  Engine queues execute in order, so
    hoisting extra waits onto same-engine NoOps (the same pattern the
    all-engine-barrier drains use) preserves semantics.  Matmults
    additionally must not carry DMA-queue-sem waits at all."""
    import concourse.mybir as mybir
    import re

    _MONO_SEM = re.compile(r"^(Pool|Activation|PE|DVE|SP|DMAHW\d|DMASW\d)_\d+$")
    n = 0
    seen = {}  # (engine, sem id) -> max value already waited on that engine
    for f in nc.m.functions:
        for b in f.blocks:
            insts = b.instructions
            out = []
            for inst in insts:
                si = inst.sync_info
                waits = list(si.on_wait) if si and si.on_wait else []
                # drop waits dominated by an earlier same-engine wait
                # (engines execute their queue in order, so sem >= v' with
                # v' >= v implies sem >= v)
                if waits:
                    eng = inst.engine
                    kept = []
                    changed = False
                    for w in waits:
                        kk = (eng, w.id)
                        monotone = bool(_MONO_SEM.match(w.ant_name or ""))
                        if (monotone
                                and getattr(w, "wait_mode", "") == "sem-ge-imm"
                                and w.wait_value is not None
                                and seen.get(kk, -1) >= w.wait_value):
                            changed = True
                            continue
                        kept.append(w)
                        if (monotone
                                and getattr(w, "wait_mode", "") == "sem-ge-imm"
                                and w.wait_value is not None):
                            seen[kk] = max(seen.get(kk, -1), w.wait_value)
                    if changed:
                        inst.sync_info = mybir.SyncInfo(
                            on_wait=kept, on_update=list(si.on_update or []))
                        si = inst.sync_info
                    waits = kept
                is_mm = type(inst).__name__ == "InstMatmult"
                mm_dma = is_mm and any(
                    "DMA" in (w.ant_name or "") for w in waits)
                keep, move = waits, []
                if len(waits) > 1 or mm_dma:
                    eng_w = [w for w in waits if "DMA" not in (w.ant_name or "")]
                    if eng_w:
                        keep = [eng_w[-1]]
                        move = [w for w in waits if w is not keep[0]]
                    else:
                        keep = []
                        move = waits
                if move:
                    for w in move:
                        nop = mybir.InstNoOp(
                            name=f"{inst.name}-lw{n}", ins=[], outs=[],
                            engine=inst.engine)
                        nop.sync_info = mybir.SyncInfo(on_wait=[w], on_update=[])
                        nc.register_instruction(nop)
                        out.append(nop)
                        n += 1
                    inst.sync_info = mybir.SyncInfo(
                        on_wait=keep, on_update=list(si.on_update or []))
                out.append(inst)
            insts[:] = out
    return nc


def _build_program_legalized(repeat=1):
    return _legalize_waits(_build_program_raw(repeat))


def _build_program_raw(repeat=1):
    import concourse.bass as bass
    import concourse.mybir as mybir
    import concourse.tile as tile

    f32 = mybir.dt.float32
    bf16 = mybir.dt.bfloat16
    OP = mybir.AluOpType
    AF = mybir.ActivationFunctionType

    nc = bass.Bass()

    x = nc.dram_tensor("x", [HW, COLS], f32, kind="ExternalInput")
    y = nc.dram_tensor("y", [HW, COLS], f32, kind="ExternalInput")
    ae8 = nc.dram_tensor("ae8", [H, 2 * W * C], f32, kind="ExternalInput")
    a8T = nc.dram_tensor("a8T", [W, H * C], f32, kind="ExternalInput")
    cf = nc.dram_tensor("cf", [128, CF_W], f32, kind="ExternalInput")
    cb = nc.dram_tensor("cb", [128, CB_W], bf16, kind="ExternalInput")
    out = nc.dram_tensor("out", [1, 2 * repeat], f32, kind="ExternalOutput")

    with tile.TileContext(nc) as tc, ExitStack() as ctx:
        consts = ctx.enter_context(tc.tile_pool(name="consts", bufs=1))
        xpool = ctx.enter_context(tc.tile_pool(name="xpool", bufs=2))
        ypool = ctx.enter_context(tc.tile_pool(name="ypool", bufs=2))
        spool = ctx.enter_context(tc.tile_pool(name="spool", bufs=2))
        small = ctx.enter_context(tc.tile_pool(name="small", bufs=2))
        acc = ctx.enter_context(tc.tile_pool(name="acc", bufs=1))
        ps_s1 = ctx.enter_context(tc.tile_pool(name="ps_s1", bufs=1, space="PSUM"))
        ps_up = ctx.enter_context(tc.tile_pool(name="ps_up", bufs=2, space="PSUM"))
        ps_sm = ctx.enter_context(tc.tile_pool(name="ps_sm", bufs=1, space="PSUM"))
        ps_pk = ctx.enter_context(tc.tile_pool(name="ps_pk", bufs=3, space="PSUM"))
        ps_mk = ctx.enter_context(tc.tile_pool(name="ps_mk", bufs=1, space="PSUM"))

        cf_sb = consts.tile([128, CF_W], f32)
        nc.gpsimd.dma_start(out=cf_sb, in_=cf[:, :])
        cb_sb = consts.tile([128, CB_W], bf16)
        nc.gpsimd.dma_start(out=cb_sb, in_=cb[:, :])

        identf_sb = cf_sb[:, CF_IDENT:CF_IDENT + 128]
        ut_sb = cf_sb[0:H, CF_UT:CF_UT + UPS]
        uht_sb = cf_sb[0:H, CF_UHT:CF_UHT + UPS].rearrange(
            "k (h m) -> k h m", h=2)
        ones_sb = cf_sb[:, CF_ONES:CF_ONES + 1]
        onesr_sb = cf_sb[0:1, CF_ONESR:CF_ONESR + 128]
        sgn_sb = cf_sb[0:1, CF_SGN:CF_SGN + C]
        dyt_sb = cb_sb[:, CB_DYT:CB_DYT + 2 * C].rearrange(
            "k (h m) -> k h m", h=2)
        dxt_sb = cb_sb[:, CB_DXT:CB_DXT + 2 * W].rearrange(
            "k (h m) -> k h m", h=2)
        identb_sb = cb_sb[0:C, CB_IDENT:CB_IDENT + C]

        ones_r = consts.tile([128, 1], mybir.dt.float32r)
        nc.scalar.copy(ones_r, ones_sb)

        for rep in range(repeat):
            out_sb = acc.tile([1, 2], f32, tag="outsb")

            # ---- cross-attn term (identical on every core) ----
            a8tt = small.tile([W, H * C], f32, tag="a8tt")
            nc.gpsimd.dma_start(out=a8tt, in_=a8T[:, :])
            ae8t = small.tile([H, 2 * W * C], f32, tag="ae8t")
            nc.gpsimd.dma_start(out=ae8t, in_=ae8[:, :])
            d8 = small.tile([H, W * C], f32, tag="d8")
            nc.vector.tensor_tensor(
                d8, ae8t[:, 0:W * C], ae8t[:, W * C:2 * W * C], OP.subtract)
            s8 = small.tile([H, W * C], f32, tag="s8")
            nc.scalar.activation(s8, d8, AF.Square)
            cross_ps = ps_sm.tile([1, W * C], f32, tag="ps")
            nc.tensor.matmul(
                cross_ps, lhsT=ones_sb[0:H, :], rhs=s8, start=True, stop=True)
            cross8 = small.tile([1, C], f32, tag="cross8")
            nc.vector.reduce_sum(
                out=cross8,
                in_=cross_ps.rearrange("p (j c) -> p c j", c=C),
                axis=mybir.AxisListType.X,
            )
            crossw = small.tile([1, C], f32, tag="crossw")
            nc.vector.tensor_tensor(crossw, cross8, sgn_sb, OP.mult)
            nc.vector.reduce_sum(
                out=out_sb[:, 1:2], in_=crossw, axis=mybir.AxisListType.X)

            # ---- mask pipeline: per-channel upsample + threshold ----
            b01 = acc.tile([128, C * 2 * UPS], bf16, tag="b01")  # [c, h, l]
            m1_sb = acc.tile([C, C * UPS], bf16, tag="m1")
            m1t_sb = acc.tile([128, 2 * C * C], bf16, tag="m1t")
            mask_ps = ps_mk.tile([C, C * W], f32, tag="mask")
            g_sb = acc.tile([C, C * W], f32, tag="g")
            w8 = acc.tile([C, W], f32, tag="w8")
            ae8_cj = ae8t[:, 0:W * C].rearrange("i (j c) -> i c j", c=C)
            a8t_ci = a8tt.rearrange("j (i c) -> j c i", c=C)
            for c in range(C):
                pk = ps_pk.tile([128, 512], f32, tag="pk")
                r_ps = pk[0:H, 64:320]
                nc.tensor.matmul(
                    r_ps, lhsT=a8t_ci[:, c, :], rhs=ut_sb,
                    start=True, stop=True)
                r_sb = small.tile([H, UPS], f32, tag="r")
                nc.scalar.copy(r_sb, r_ps)
                up_ps = ps_up.tile([128, 2 * UPS], f32, tag="up")
                for h in range(2):
                    nc.tensor.matmul(
                        up_ps[:, h * UPS:(h + 1) * UPS],
                        lhsT=uht_sb[:, h, :], rhs=r_sb, start=True, stop=True,
                    )
                mxc = small.tile([128, 1], f32, tag="mxc")
                nc.vector.reduce_max(out=mxc, in_=up_ps, axis=mybir.AxisListType.X)
                mxr_ps = pk[0:1, 320:448]
                nc.tensor.transpose(mxr_ps, mxc, identf_sb)
                mxs = small.tile([1, 1], f32, tag="mxs")
                nc.vector.reduce_max(out=mxs, in_=mxr_ps, axis=mybir.AxisListType.X)
                ts_sb = small.tile([1, 1], f32, tag="ts")
                nc.vector.tensor_scalar_mul(ts_sb, in0=mxs, scalar1=float(THRS[c]))
                tb_ps = pk[0:128, 448:449]
                nc.tensor.matmul(
                    tb_ps, lhsT=onesr_sb, rhs=ts_sb, start=True, stop=True)
                tthr = small.tile([128, 1], f32, tag="tthr")
                nc.vector.tensor_copy(tthr, tb_ps)
                nc.vector.tensor_scalar(
                    out=b01[:, c * 2 * UPS:(c + 1) * 2 * UPS],
                    in0=up_ps,
                    scalar1=tthr, scalar2=None, op0=OP.is_ge,
                )
                m1_ps = ps_sm.tile([C, UPS], f32, tag="ps")
                for h in range(2):
                    nc.tensor.matmul(
                        m1_ps, lhsT=dyt_sb[:, h, :],
                        rhs=b01[:, c * 2 * UPS + h * UPS:
                                c * 2 * UPS + (h + 1) * UPS],
                        start=(h == 0), stop=(h == 1),
                    )
                nc.vector.tensor_copy(m1_sb[:, c * UPS:(c + 1) * UPS], m1_ps)
                for h in range(2):
                    tr_ps = ps_sm.tile([128, C], bf16, tag="ps")
                    nc.tensor.transpose(
                        tr_ps,
                        m1_sb[0:C, c * UPS + h * 128: c * UPS + (h + 1) * 128],
                        identb_sb,
                    )
                    k = (2 * c + h) * C
                    nc.vector.tensor_copy(m1t_sb[:, k:k + C], tr_ps)
                for h in range(2):
                    k = (2 * c + h) * C
                    nc.tensor.matmul(
                        mask_ps[:, c * W:(c + 1) * W],
                        lhsT=m1t_sb[:, k:k + C],
                        rhs=dxt_sb[:, h, :],
                        start=(h == 0), stop=(h == 1),
                    )
                nc.vector.tensor_scalar(
                    out=g_sb[:, c * W:(c + 1) * W],
                    in0=mask_ps[:, c * W:(c + 1) * W],
                    scalar1=0.0, scalar2=None, op0=OP.is_gt)
                if c == 0:
                    nc.gpsimd.tensor_copy(w8, g_sb[:, 0:W])
                else:
                    nc.gpsimd.tensor_tensor(
                        w8, w8, g_sb[:, c * W:(c + 1) * W],
                        OP.add if SGNS[c] > 0 else OP.subtract)

            # signed per-column gate row in the s1 layout: wrow[0, r*64+w]
            # = w8[r, w].  Partition-fold via 8 tiny PE selector matmuls
            # (lhsT = identity column r) -- no DMA, runs mid-stream on PE.
            wrow_pk = ps_pk.tile([128, 512], f32, tag="pk")
            wrow_ps = wrow_pk[0:1, 0:COLS]
            for r in range(C):
                nc.tensor.matmul(
                    wrow_ps[:, r * W:(r + 1) * W],
                    lhsT=identf_sb[0:C, r:r + 1], rhs=w8,
                    start=True, stop=True)
            wrow = acc.tile([1, COLS], f32, tag="wrow")
            nc.scalar.copy(wrow, wrow_ps)

            # ---- main streaming loop: column sums of (x-y)^2 ----
            # Layout "(q s) j": partition q holds SUB consecutive DRAM rows,
            # so each partition's slice is one contiguous 2048*SUB-byte
            # descriptor (vs 2048 B with the interleaved layout).  All x
            # chunks stream on the SP HWDGE ring, all y chunks on the ACT
            # HWDGE ring; SWDGE (Pool) carries only small/const DMAs.
            s1 = ps_s1.tile([1, COLS], f32, tag="s1")
            n_mm = sum(SUBS)
            mm = 0
            off = 0
            pr = 0
            for t, SUB in enumerate(SUBS):
                xt = xpool.tile([128, SUB, COLS], f32, tag="xt")
                yt = ypool.tile([128, SUB, COLS], f32, tag="yt")
                nc.sync.dma_start(
                    out=xt, in_=x[off:off + 128 * SUB, :].rearrange(
                        "(q s) j -> q s j", q=128))
                nc.sync.dma_start(
                    out=yt, in_=y[off:off + 128 * SUB, :].rearrange(
                        "(q s) j -> q s j", q=128))
                off += 128 * SUB
                st = spool.tile([128, SUB, COLS], mybir.dt.float32r, tag="s")
                if t >= len(SUBS) - DRAIN_CHUNKS:
                    # drain region: split every op by columns across engine
                    # pairs to minimise latency after each DMA lands
                    hc = COLS // 2
                    for s in range(SUB):
                        nc.vector.tensor_tensor(
                            xt[:, s, 0:hc], xt[:, s, 0:hc], yt[:, s, 0:hc],
                            OP.subtract)
                        nc.gpsimd.tensor_tensor(
                            xt[:, s, hc:COLS], xt[:, s, hc:COLS],
                            yt[:, s, hc:COLS], OP.subtract)
                        nc.scalar.activation(
                            st[:, s, 0:hc], xt[:, s, 0:hc], AF.Square)
                        nc.vector.tensor_tensor(
                            st[:, s, hc:COLS], xt[:, s, hc:COLS],
                            xt[:, s, hc:COLS], OP.mult)
                        nc.tensor.matmul(
                            s1, lhsT=ones_r, rhs=st[:, s, :],
                            start=(mm == 0), stop=(mm == n_mm - 1),
                            skip_group_check=True,
                        )
                        mm += 1
                    continue
                q = 0
                while q < SUB:
                    qw = min(2, SUB - q)
                    sub_eng = nc.gpsimd
                    sub_eng.tensor_tensor(
                        xt[:, q:q + qw, :], xt[:, q:q + qw, :],
                        yt[:, q:q + qw, :], OP.subtract)
                    nc.scalar.activation(
                        st[:, q:q + qw, :], xt[:, q:q + qw, :], AF.Square)
                    for s in range(q, q + qw):
                        nc.tensor.matmul(
                            s1, lhsT=ones_r,
                            rhs=st[:, s, :],
                            start=(mm == 0), stop=(mm == n_mm - 1),
                            skip_group_check=True,
                        )
                        mm += 1
                    pr += 1
                    q += qw

            # ---- dot with the signed gate row; tail is two DVE ops ----
            s1w = acc.tile([1, COLS], f32, tag="s1w")
            nc.vector.tensor_tensor(s1w, s1, wrow, OP.mult)
            nc.vector.reduce_sum(
                out=out_sb[:, 0:1], in_=s1w, axis=mybir.AxisListType.X)
            nc.sync.dma_start(out=out[:, 2 * rep:2 * rep + 2], in_=out_sb)

    return nc


def _build_program(repeat=1):
    return _build_program_legalized(repeat)


def _get_program(repeat=1):
    key = ("nc", repeat)
    if key not in _PROG_CACHE:
        _PROG_CACHE[key] = _build_program(repeat)
    return _PROG_CACHE[key]


def _host_constants():
    if "consts" in _PROG_CACHE:
        return _PROG_CACHE["consts"]
    from concourse import mybir

    np_bf16 = mybir.dt.np(mybir.dt.bfloat16)
    u = _interp_matrix(UPS, H)          # (256, 64) upsample
    d = _interp_matrix(W, UPS)          # (64, 256) downsample, weights in {0,.5}

    cfh = np.zeros((128, CF_W), dtype=np.float32)
    cfh[:, CF_IDENT:CF_IDENT + 128] = np.eye(128, dtype=np.float32)
    cfh[0:H, CF_UT:CF_UT + UPS] = u.T
    cfh[0:H, CF_UHT:CF_UHT + UPS] = u.T.reshape(H, UPS)  # same data as ut;
    # uht view k,(h m): U[h*128+m, k] == u.T[k, h*128+m] -> identical layout
    cfh[:, CF_ONES] = 1.0
    cfh[0, CF_ONESR:CF_ONESR + 128] = 1.0
    cfh[0, CF_SGN:CF_SGN + C] = np.asarray(SGNS, dtype=np.float32)

    cbs = []
    dxt = d.T.reshape(2, 128, W).transpose(1, 0, 2).reshape(128, 2 * W)
    for core in range(NCORES):
        cbh = np.zeros((128, CB_W), dtype=np.float32)
        dsl = d[core * C:(core + 1) * C, :]  # (8, 256)
        cbh[:, CB_DYT:CB_DYT + 2 * C] = (
            dsl.T.reshape(2, 128, C).transpose(1, 0, 2).reshape(128, 2 * C))
        cbh[:, CB_DXT:CB_DXT + 2 * W] = dxt
        cbh[0:C, CB_IDENT:CB_IDENT + C] = np.eye(C, dtype=np.float32)
        cbs.append(np.ascontiguousarray(cbh).astype(np_bf16))

    consts = {"cf": cfh, "cbs": cbs}
    _PROG_CACHE["consts"] = consts
    return consts


def _make_in_maps(inputs):
    attn = np.ascontiguousarray(inputs["attn"], dtype=np.float32)
    attn_erase = np.ascontiguousarray(inputs["attn_erase"], dtype=np.float32)
    sa = np.ascontiguousarray(
        inputs["self_attn"], dtype=np.float32).reshape(HW, HW)
    sae = np.ascontiguousarray(
        inputs["self_attn_erase"], dtype=np.float32).reshape(HW, HW)

    a8 = np.ascontiguousarray(attn[:, :, 1:1 + C]).reshape(H, W * C)
    a8t_host = np.ascontiguousarray(
        attn[:, :, 1:1 + C].transpose(1, 0, 2)).reshape(W, H * C)
    e8 = np.ascontiguousarray(attn_erase[:, :, 1:1 + C]).reshape(H, W * C)
    ae8 = np.concatenate([a8, e8], axis=1)
    ch = _host_constants()

    in_maps = []
    for core in range(NCORES):
        in_maps.append({
            "x": np.ascontiguousarray(sa[:, core * COLS:(core + 1) * COLS]),
            "y": np.ascontiguousarray(sae[:, core * COLS:(core + 1) * COLS]),
            "ae8": ae8,
            "a8T": a8t_host,
            "cf": ch["cf"],
            "cb": ch["cbs"][core],
        })
    return in_maps


def _combine(outs):
    self_raw = sum(float(o[0, 0]) for o in outs)
    cross_raw = float(outs[0][0, 1])
    return np.float32((self_raw + cross_raw) / float(HW))


def kernel(**inputs):
    from concourse.bass_utils import run_bass_kernel_spmd

    nc = _get_program()
    in_maps = _make_in_maps(inputs)
    res = run_bass_kernel_spmd(nc, in_maps, core_ids=list(range(NCORES)))
    return _combine([r["out"] for r in res.results])

